# revision 19
# baseline (speedup 1.0000x reference)
"""GCNEncoder (3x GraphConv, D=64) on 8 Trainium2 NeuronCores.

Strategy:
  - Host: dedup edges, relabel nodes by in-degree (descending), partition the
    relabeled dst nodes into 128-row blocks dealt round-robin across 8 cores,
    and build a block-ELL structure (per dst-block: K_j neighbor slots per
    node, uniform across cores so a single SPMD program works).
  - Linearity: agg @ W_rel == segment_sum(w * (h @ W_rel)[src]), so each layer
    keeps a node-major table y = h @ W_rel in HBM, and the aggregation output
    plus the root term r = h @ W_root + b is already the layer output.
  - Device, per layer: per dst-block, an indirect DMA gathers the K_j neighbor
    rows per partition from the y table; DVE multiplies by edge weights
    (broadcast along features) and does a strided reduce over K; add the
    resident r term; ReLU + two 64x64 matmuls produce the next layer's y/r;
    an AllGather rebuilds the full y table between layers.
  - Transfers are minimized: each core receives only its x shard (bf16), a
    compact [16, L] token table (replicated to 128 partitions on device),
    bf16 edge weights (converted on device), and the 64x64 weight matrices;
    the layer-1 dense part is computed on device and the output is returned
    in bf16.
"""

import os

import numpy as np

P = 128
D = 64
NCORES = 8


# ---------------------------------------------------------------- host prep


def _preprocess(x, edge_index, edge_weight):
    N = x.shape[0]
    src = np.asarray(edge_index[0], dtype=np.int64)
    dst = np.asarray(edge_index[1], dtype=np.int64)
    w = np.asarray(edge_weight, dtype=np.float64)

    # dedup parallel edges (sum weights)
    key = dst * N + src
    ukey, inv = np.unique(key, return_inverse=True)
    uw = np.bincount(inv, weights=w).astype(np.float32)
    udst = (ukey // N).astype(np.int64)
    usrc = (ukey % N).astype(np.int64)

    deg = np.bincount(udst, minlength=N)

    # per-core block count
    B = -(-N // (NCORES * P))  # ceil
    Npad = NCORES * B * P

    # order nodes by degree desc; sorted position t -> orig node order[t]
    order = np.argsort(-deg, kind="stable")
    order_pad = np.concatenate([order, np.full(Npad - N, -1, dtype=np.int64)])

    # sorted block g = j*NCORES + c  ->  core c, slot j
    # new id layout: new = c*B*P + j*P + p  where sorted pos t = g*P + p
    t = np.arange(Npad)
    g = t // P
    p = t % P
    c = g % NCORES
    j = g // NCORES
    newpos_of_sorted = c * (B * P) + j * P + p
    # perm: new id -> orig node (-1 for dummy)
    perm = np.empty(Npad, dtype=np.int64)
    perm[newpos_of_sorted] = order_pad
    # inv_new: orig node -> new id
    sorted_pos = np.empty(N, dtype=np.int64)
    sorted_pos[order] = np.arange(N)
    inv_new = newpos_of_sorted[sorted_pos]

    # dma_gather indices are signed int16, so the table is addressed through
    # four 32768-row windows; per (block slot j, window w) the neighbor count
    # is padded to the max over all cores/dsts of that slot (uniform SPMD).
    WIN = 32768
    NW = -(-Npad // WIN)
    nd = inv_new[udst]  # new dst id per edge
    ns = inv_new[usrc]  # new src id per edge
    wid = ns // WIN

    ej_all = (nd % (B * P)) // P
    ep_all = nd % P
    ec_all = nd // (B * P)
    # counts per (core, slot j, partition, window)
    cnt = np.zeros((NCORES, B, P, NW), dtype=np.int64)
    np.add.at(cnt, (ec_all, ej_all, ep_all, wid), 1)
    K_jw = cnt.max(axis=(0, 2))  # [B, NW]
    if K_jw.sum() == 0:
        K_jw[:, 0] = 1
    # ensure at least one column per block (so g tile is non-empty)
    K_jw[:, 0] = np.maximum(K_jw[:, 0], 1)
    K_j = K_jw.sum(axis=1)  # [B] total columns per block
    off_j = np.concatenate([[0], np.cumsum(K_j)])
    off_jw = np.concatenate(
        [np.zeros((B, 1), np.int64), np.cumsum(K_jw, axis=1)], axis=1
    ) + off_j[:-1, None]
    K_total = int(off_j[-1])

    # rank of each edge within its (dst, window) group
    gkey = nd * NW + wid
    eorder = np.argsort(gkey, kind="stable")
    gk_s = gkey[eorder]
    nd_s = nd[eorder]
    wid_s = wid[eorder]
    ns_s = ns[eorder]
    w_s = uw[eorder]
    first = np.concatenate([[True], gk_s[1:] != gk_s[:-1]])
    gid = np.cumsum(first) - 1
    gstart = np.nonzero(first)[0]
    k_within = np.arange(len(gk_s)) - gstart[gid]

    ec = nd_s // (B * P)
    rem = nd_s % (B * P)
    ej = rem // P
    ep = rem % P
    col = off_jw[ej, wid_s] + k_within

    ell_idx = np.zeros((NCORES, P, K_total), dtype=np.int16)  # window-local
    ell_w = np.zeros((NCORES, P, K_total), dtype=np.float32)
    ell_idx[ec, ep, col] = (ns_s % WIN).astype(np.int16)
    ell_w[ec, ep, col] = w_s

    # token-format (wrapped int16) index arrays for dma_gather, compact
    # [16, ntok/16] form (the on-device kernel replicates to 128 partitions):
    ntok_jw = K_jw * P
    tok_cum = np.concatenate([[0], np.cumsum(ntok_jw.reshape(-1))])
    TOK_TOTAL = int(tok_cum[-1])
    idx_tok = np.zeros((NCORES, 16, TOK_TOTAL // 16), dtype=np.int16)
    for j in range(B):
        for w in range(NW):
            K = int(K_jw[j, w])
            if K == 0:
                continue
            c0 = int(off_jw[j, w])  # absolute col start
            t0 = int(tok_cum[j * NW + w])
            ntok = K * P
            # tokens [K, P] -> linear (c*128+p) -> wrap [ntok/16, 16] -> T
            blk = ell_idx[:, :, c0 : c0 + K]  # [NCORES, P, K]
            lin = blk.transpose(0, 2, 1).reshape(NCORES, ntok)  # t = c*128+p
            wrapped = lin.reshape(NCORES, ntok // 16, 16).transpose(0, 2, 1)
            idx_tok[:, :, t0 // 16 : (t0 + ntok) // 16] = wrapped

    # per-core x shard, node-major new layout: x_sh[c][j*P+p, f], quantized
    # to int8 with a per-row scale (shipped as [128, B] f32: xs[p, j]).
    xf = np.asarray(x, dtype=np.float32)
    x_new = np.zeros((Npad, D), dtype=np.float32)
    real = perm >= 0
    x_new[real] = xf[perm[real]]
    x_sh = x_new.reshape(NCORES, B * P, D)
    xs_row = np.maximum(np.abs(x_sh).max(axis=2), 1e-20) / 127.0  # [NC, B*P]
    x_q = np.rint(x_sh / xs_row[:, :, None]).astype(np.int8)
    xs = np.ascontiguousarray(
        xs_row.reshape(NCORES, B, P).transpose(0, 2, 1)
    )  # [NC, 128, B]

    return dict(
        N=N,
        B=B,
        Npad=Npad,
        WIN=WIN,
        NW=NW,
        perm=perm,
        K_j=K_j,
        off_j=off_j,
        K_jw=K_jw,
        off_jw=off_jw,
        tok_cum=tok_cum,
        TOK_TOTAL=TOK_TOTAL,
        K_total=K_total,
        idx_tok=idx_tok,
        ell_w=ell_w,
        x_q=x_q,
        xs=xs,
    )


# ---------------------------------------------------------------- bass build


def _build(prep):
    import concourse.bacc as bacc
    import concourse.mybir as mybir
    import concourse.tile as tile
    from concourse.masks import make_identity

    f32 = mybir.dt.float32
    bf16 = mybir.dt.bfloat16
    i16 = mybir.dt.int16
    i8 = mybir.dt.int8
    B = prep["B"]
    Npad = prep["Npad"]
    WIN = prep["WIN"]
    NW = prep["NW"]
    K_j = prep["K_j"]
    off_j = prep["off_j"]
    K_jw = prep["K_jw"]
    off_jw = prep["off_jw"]
    tok_cum = prep["tok_cum"]
    TOK_TOTAL = prep["TOK_TOTAL"]
    K_total = prep["K_total"]
    L = TOK_TOTAL // 16

    nc = bacc.Bacc(
        "TRN2",
        target_bir_lowering=False,
        debug=False,
        num_devices=NCORES,
    )

    # IO
    x_in = nc.dram_tensor("x_q", [B * P, D], i8, kind="ExternalInput")
    xs_in = nc.dram_tensor("xs", [P, B], f32, kind="ExternalInput")
    idx_in = nc.dram_tensor("idx_tok", [16, L], i16, kind="ExternalInput")
    w_in = nc.dram_tensor("ell_w", [P, K_total], bf16, kind="ExternalInput")
    wmat_in = {}
    for nm in ("W_rel1", "W_root1", "W_rel2", "W_root2", "W_rel3", "W_root3"):
        wmat_in[nm] = nc.dram_tensor(nm, [D, D], f32, kind="ExternalInput")
    b_in = {}
    for nm in ("b1", "b2", "b3"):
        b_in[nm] = nc.dram_tensor(nm, [1, D], f32, kind="ExternalInput")
    out_t = nc.dram_tensor("h3q", [B * P, D], i8, kind="ExternalOutput")
    outs_t = nc.dram_tensor("h3s", [P, B], f32, kind="ExternalOutput")

    with tile.TileContext(nc) as tc:
        with (
            tc.tile_pool(name="const", bufs=1) as cpool,
            tc.tile_pool(name="dram", bufs=1, space="DRAM") as dpool,
            tc.tile_pool(name="gather", bufs=4) as gpool,
            tc.tile_pool(name="work", bufs=4) as wpool,
            tc.tile_pool(name="psum", bufs=1, space="PSUM") as ppool,
        ):
            # residents
            idx_res = cpool.tile([P, L], i16, tag="idx")
            w_res = cpool.tile([P, K_total], f32, tag="w")
            r_res = cpool.tile([P, B * D], f32, tag="r")
            xs_res = cpool.tile([P, B], f32, tag="xs")
            os_res = cpool.tile([P, B], f32, tag="os")
            ident = cpool.tile([P, P], f32, tag="ident")
            Wt = {k: cpool.tile([D, D], f32, tag=k, name=k) for k in wmat_in}
            bt = {k: cpool.tile([1, D], f32, tag=k, name=k) for k in b_in}
            b_bc = {
                k: cpool.tile([P, D], f32, tag=k + "bc", name=k + "bc") for k in b_in
            }

            # replicate the [16, L] token table to all 128 partitions
            for k8 in range(8):
                nc.sync.dma_start(
                    out=idx_res[16 * k8 : 16 * (k8 + 1), :], in_=idx_in.ap()
                )
            # edge weights: bf16 -> f32 resident
            w_bf = wpool.tile([P, K_total], bf16, tag="wbf")
            nc.sync.dma_start(out=w_bf[:], in_=w_in.ap())
            nc.vector.tensor_copy(out=w_res[:], in_=w_bf[:])
            nc.sync.dma_start(out=xs_res[:], in_=xs_in.ap())
            for k in Wt:
                nc.sync.dma_start(out=Wt[k][:], in_=wmat_in[k].ap())
            for k in bt:
                nc.sync.dma_start(out=bt[k][:], in_=b_in[k].ap())
                nc.gpsimd.partition_broadcast(b_bc[k][:], bt[k][:])
            make_identity(nc, ident[:])

            # DRAM: ping-pong table + own-shard staging
            table2 = dpool.tile([Npad, D], f32, tag="table")
            y_own = dpool.tile([B * P, D], f32, tag="yown")

            # ---- layer-0 dense pass: y_own = x @ W_rel1, r = x @ W_root1
            for jb in range(B):
                x_q8 = wpool.tile([P, D], i8, tag="xq8")
                nc.sync.dma_start(
                    out=x_q8[:], in_=x_in.ap()[jb * P : (jb + 1) * P, :]
                )
                x_f = wpool.tile([P, D], f32, tag="xf")
                nc.vector.tensor_scalar(
                    out=x_f[:],
                    in0=x_q8[:],
                    scalar1=xs_res[:, jb : jb + 1],
                    scalar2=None,
                    op0=mybir.AluOpType.mult,
                )
                xTp = ppool.tile([D, P], f32, tag="hTp", bufs=2)
                nc.tensor.transpose(out=xTp[:], in_=x_f[:], identity=ident[:])
                xT = wpool.tile([D, P], f32, tag="hT")
                nc.scalar.activation(
                    out=xT[:], in_=xTp[:], func=mybir.ActivationFunctionType.Copy
                )
                yp = ppool.tile([P, D], f32, tag="yp", bufs=2)
                nc.tensor.matmul(
                    out=yp[:], lhsT=xT[:], rhs=Wt["W_rel1"][:], start=True, stop=True
                )
                rp = ppool.tile([P, D], f32, tag="rp", bufs=2)
                nc.tensor.matmul(
                    out=rp[:], lhsT=xT[:], rhs=Wt["W_root1"][:], start=True, stop=True
                )
                y_s = wpool.tile([P, D], f32, tag="y_s")
                nc.scalar.activation(
                    out=y_s[:], in_=yp[:], func=mybir.ActivationFunctionType.Copy
                )
                nc.vector.tensor_copy(out=r_res[:, jb * D : (jb + 1) * D], in_=rp[:])
                nc.sync.dma_start(out=y_own[jb * P : (jb + 1) * P, :], in_=y_s[:])
            # r += b1 (broadcast across blocks)
            nc.vector.tensor_tensor(
                out=r_res[:].rearrange("p (j f) -> p j f", f=D),
                in0=r_res[:].rearrange("p (j f) -> p j f", f=D),
                in1=b_bc["b1"][:].unsqueeze(1).to_broadcast([P, B, D]),
                op=mybir.AluOpType.add,
            )

            for layer in (1, 2, 3):
                nc.gpsimd.collective_compute(
                    "AllGather",
                    mybir.AluOpType.bypass,
                    replica_groups=[list(range(NCORES))],
                    ins=[y_own[:].opt()],
                    outs=[table2[:].opt()],
                )
                W_rel_nxt = Wt[f"W_rel{layer + 1}"] if layer < 3 else None
                W_root_nxt = Wt[f"W_root{layer + 1}"] if layer < 3 else None

                for jb in range(B):
                    K = int(K_j[jb])
                    off = int(off_j[jb])
                    g = gpool.tile([P, K * D], f32, tag="g")
                    # one dma_gather per 32768-row table window
                    for wnd in range(NW):
                        Kw = int(K_jw[jb, wnd])
                        if Kw == 0:
                            continue
                        cw = int(off_jw[jb, wnd]) - off
                        ntok = Kw * P
                        t0 = int(tok_cum[jb * NW + wnd])
                        r0 = wnd * WIN
                        r1 = min(Npad, (wnd + 1) * WIN)
                        nc.gpsimd.dma_gather(
                            out_ap=g[:, cw * D : (cw + Kw) * D].rearrange(
                                "p (c e) -> p c e", e=D
                            ),
                            in_ap=table2[:][r0:r1, :],
                            idxs_ap=idx_res[:, t0 // 16 : (t0 + ntok) // 16],
                            num_idxs=ntok,
                            num_idxs_reg=ntok,
                            elem_size=D,
                            single_packet=False,
                        )
                    # g *= w (broadcast along feature dim)
                    g3 = g[:].rearrange("p (k f) -> p k f", f=D)
                    wb = w_res[:, off : off + K].unsqueeze(-1).to_broadcast([P, K, D])
                    nc.vector.tensor_tensor(
                        out=g3, in0=g3, in1=wb, op=mybir.AluOpType.mult
                    )
                    # agg[p, f] = sum_k g[p, k, f]
                    agg = wpool.tile([P, D], f32, tag="agg")
                    gT = g[:].rearrange("p (k f) -> p f k", f=D)
                    nc.vector.reduce_sum(
                        out=agg[:], in_=gT, axis=mybir.AxisListType.X
                    )
                    # pre = agg + r
                    pre = wpool.tile([P, D], f32, tag="pre")
                    nc.vector.tensor_add(
                        out=pre[:],
                        in0=agg[:],
                        in1=r_res[:, jb * D : (jb + 1) * D],
                    )

                    if layer == 3:
                        # int8 quantize with per-row (per-partition) scale
                        am = wpool.tile([P, 1], f32, tag="am")
                        nc.vector.reduce_max(
                            out=am[:],
                            in_=pre[:],
                            axis=mybir.AxisListType.X,
                            apply_absolute_value=True,
                        )
                        nc.vector.tensor_scalar_max(am[:], am[:], 1e-20)
                        inv = wpool.tile([P, 1], f32, tag="inv")
                        nc.vector.reciprocal(out=inv[:], in_=am[:])
                        nc.vector.tensor_scalar_mul(inv[:], inv[:], 127.0)
                        o_q = wpool.tile([P, D], i8, tag="oq")
                        nc.vector.tensor_scalar(
                            out=o_q[:],
                            in0=pre[:],
                            scalar1=inv[:, 0:1],
                            scalar2=None,
                            op0=mybir.AluOpType.mult,
                        )
                        nc.vector.tensor_scalar_mul(
                            os_res[:, jb : jb + 1], am[:], 1.0 / 127.0
                        )
                        nc.sync.dma_start(
                            out=out_t.ap()[jb * P : (jb + 1) * P, :], in_=o_q[:]
                        )
                        continue

                    # h = relu(pre); hT via PE transpose
                    h = wpool.tile([P, D], f32, tag="h")
                    nc.scalar.activation(
                        out=h[:], in_=pre[:], func=mybir.ActivationFunctionType.Relu
                    )
                    hTp = ppool.tile([D, P], f32, tag="hTp", bufs=2)
                    nc.tensor.transpose(out=hTp[:], in_=h[:], identity=ident[:])
                    hT = wpool.tile([D, P], f32, tag="hT")
                    nc.scalar.activation(
                        out=hT[:], in_=hTp[:], func=mybir.ActivationFunctionType.Copy
                    )
                    # y = h @ W_rel ; r = h @ W_root, node-major via lhsT=hT
                    yp = ppool.tile([P, D], f32, tag="yp", bufs=2)
                    nc.tensor.matmul(
                        out=yp[:], lhsT=hT[:], rhs=W_rel_nxt[:], start=True, stop=True
                    )
                    rp = ppool.tile([P, D], f32, tag="rp", bufs=2)
                    nc.tensor.matmul(
                        out=rp[:], lhsT=hT[:], rhs=W_root_nxt[:], start=True, stop=True
                    )
                    y_s = wpool.tile([P, D], f32, tag="y_s")
                    nc.scalar.activation(
                        out=y_s[:], in_=yp[:], func=mybir.ActivationFunctionType.Copy
                    )
                    nc.vector.tensor_copy(
                        out=r_res[:, jb * D : (jb + 1) * D], in_=rp[:]
                    )
                    nc.sync.dma_start(
                        out=y_own[jb * P : (jb + 1) * P, :], in_=y_s[:]
                    )

                if layer < 3:
                    # r += b (broadcast across blocks)
                    nc.vector.tensor_tensor(
                        out=r_res[:].rearrange("p (j f) -> p j f", f=D),
                        in0=r_res[:].rearrange("p (j f) -> p j f", f=D),
                        in1=b_bc[f"b{layer + 1}"][:]
                        .unsqueeze(1)
                        .to_broadcast([P, B, D]),
                        op=mybir.AluOpType.add,
                    )

            nc.sync.dma_start(out=outs_t.ap(), in_=os_res[:])

    nc.compile()
    return nc


# ---------------------------------------------------------------- entry


def _prep_and_build(inputs):
    import ml_dtypes

    prep = _preprocess(inputs["x"], inputs["edge_index"], inputs["edge_weight"])
    nc = _build(prep)
    W = {
        k: np.ascontiguousarray(np.asarray(inputs[k], dtype=np.float32))
        for k in (
            "W_rel1",
            "b_rel1",
            "W_root1",
            "W_rel2",
            "b_rel2",
            "W_root2",
            "W_rel3",
            "b_rel3",
            "W_root3",
        )
    }
    in_maps = []
    for c in range(NCORES):
        in_maps.append(
            {
                "x_q": prep["x_q"][c],
                "xs": prep["xs"][c],
                "idx_tok": np.ascontiguousarray(prep["idx_tok"][c]),
                "ell_w": prep["ell_w"][c].astype(ml_dtypes.bfloat16),
                "W_rel1": W["W_rel1"],
                "W_root1": W["W_root1"],
                "W_rel2": W["W_rel2"],
                "W_root2": W["W_root2"],
                "W_rel3": W["W_rel3"],
                "W_root3": W["W_root3"],
                "b1": W["b_rel1"].reshape(1, D),
                "b2": W["b_rel2"].reshape(1, D),
                "b3": W["b_rel3"].reshape(1, D),
            }
        )
    return prep, nc, in_maps


def _reassemble(prep, core_outs):
    N = prep["N"]
    B = prep["B"]
    perm = prep["perm"]
    out = np.zeros((N, D), dtype=np.float32)
    for c in range(NCORES):
        q, s = core_outs[c]  # [B*P, D] int8, [P, B] f32
        vals = q.astype(np.float32) * s.T.reshape(B * P, 1)
        pr = perm[c * B * P : (c + 1) * B * P]
        real = pr >= 0
        out[pr[real]] = vals[real]
    return out


def _make_runner(nc):
    """Build a callable(in_maps) -> per-core output dicts that executes the
    compiled Bass program via PJRT on the 8 NeuronCores.

    Mirrors concourse.bass2jax.run_bass_via_pjrt's multi-core branch, except
    the donated ExternalOutput zero-buffers are created ON DEVICE (jitted
    fill with sharded output) instead of uploading host zeros through the
    axon tunnel each call.
    """
    import jax
    import jax.numpy as jnp
    from jax.experimental.shard_map import shard_map
    from jax.sharding import Mesh, NamedSharding, PartitionSpec

    import concourse.mybir as mybir
    from concourse.bass2jax import (
        _bass_exec_p,
        install_neuronx_cc_hook,
        partition_id_tensor,
    )

    install_neuronx_cc_hook()
    n_cores = NCORES
    partition_name = nc.partition_id_tensor.name if nc.partition_id_tensor else None

    in_names = []
    out_names = []
    out_avals = []
    for alloc in nc.m.functions[0].allocations:
        if not isinstance(alloc, mybir.MemoryLocationSet):
            continue
        if alloc.kind not in ("ExternalInput", "ExternalOutput"):
            continue
        name = alloc.memorylocations[0].name
        if alloc.kind == "ExternalInput":
            if name != partition_name:
                in_names.append(name)
        else:
            out_names.append(name)
            out_avals.append(
                jax.core.ShapedArray(
                    tuple(alloc.tensor_shape), mybir.dt.np(alloc.dtype)
                )
            )
    n_params = len(in_names)
    n_outs = len(out_names)
    all_names = list(in_names) + out_names
    if partition_name is not None:
        all_names.append(partition_name)

    def _body(*args):
        operands = list(args)
        if partition_name is not None:
            operands.append(partition_id_tensor())
        return tuple(
            _bass_exec_p.bind(
                *operands,
                out_avals=tuple(out_avals),
                in_names=tuple(all_names),
                out_names=tuple(out_names),
                lowering_input_output_aliases=(),
                sim_require_finite=True,
                sim_require_nnan=True,
                nc=nc,
            )
        )

    devices = jax.devices()[:n_cores]
    mesh = Mesh(np.asarray(devices), ("core",))
    in_specs = (PartitionSpec("core"),) * (n_params + n_outs)
    out_specs = (PartitionSpec("core"),) * n_outs
    donate = tuple(range(n_params, n_params + n_outs))
    sharded = jax.jit(
        shard_map(
            _body, mesh=mesh, in_specs=in_specs, out_specs=out_specs, check_rep=False
        ),
        donate_argnums=donate,
        keep_unused=True,
    )
    zshard = NamedSharding(mesh, PartitionSpec("core"))

    def _mkzeros():
        return tuple(
            jnp.zeros((n_cores * av.shape[0], *av.shape[1:]), av.dtype)
            for av in out_avals
        )

    mkzeros = jax.jit(_mkzeros, out_shardings=(zshard,) * n_outs)

    timeit = bool(int(os.environ.get("GCN_TIMEIT", "0")))

    def run(in_maps):
        import time as _time

        t0 = _time.time()
        concat_in = [
            np.concatenate(
                [np.asarray(in_maps[c][name]) for c in range(n_cores)], axis=0
            )
            for name in in_names
        ]
        t1 = _time.time()
        zs = mkzeros()
        out_arrs = sharded(*concat_in, *zs)
        for o in out_arrs:
            o.block_until_ready()
        t2 = _time.time()
        hosted = [np.asarray(o) for o in out_arrs]
        t3 = _time.time()
        if timeit:
            print(
                f"[runner] concat {t1 - t0:.3f}s  dispatch+exec {t2 - t1:.3f}s  "
                f"fetch {t3 - t2:.3f}s"
            )
        return [
            {
                name: hosted[i].reshape(n_cores, *out_avals[i].shape)[c]
                for i, name in enumerate(out_names)
            }
            for c in range(n_cores)
        ]

    return run


def kernel(**inputs) -> np.ndarray:
    prep, nc, in_maps = _prep_and_build(inputs)
    run = _make_runner(nc)
    results = run(in_maps)
    kernel.last_run = run
    kernel.last_nc = nc
    kernel.last_in_maps = in_maps
    return _reassemble(
        prep, [(results[c]["h3q"], results[c]["h3s"]) for c in range(NCORES)]
    )


if __name__ == "__main__":
    import reference

    inputs = {k: np.asarray(v) for k, v in reference.setup_inputs().items()}
    expected = np.asarray(reference.reference(**inputs))
    actual = kernel(**inputs)
    err = np.abs(actual - expected).max() / (np.abs(expected).max() + 1e-9)
    rel = np.linalg.norm(actual - expected) / (np.linalg.norm(expected) + 1e-30)
    print("max-abs-rel:", err, " fro-rel:", rel)


# revision 23
# speedup vs baseline: 1.3760x; 1.3760x over previous
"""GCNEncoder (3x GraphConv, D=64) on 8 Trainium2 NeuronCores.

The end-to-end call is dominated by the axon tunnel (host<->device ~55-75
MB/s, plus fixed per-array costs), so the design minimizes wire bytes and
array count:

  - ONE flat int16 input per core packing: dma_gather index tokens (i16),
    per-token weights (bf16), per-token dst-partition ids (i8), the int8
    row-quantized x shard + f32 row scales, the six 64x64 weight matrices,
    biases, and an iota row. Regions are unpacked on device with SBUF-side
    bitcasts.
  - ONE flat int8 output per core packing the int8 row-quantized result and
    its f32 row scales.
  - Aggregation is token-exact (no ELL row padding): edges are packed
    densely per (dst-block, src-window), padded only to 128-token chunks и
    to the max count over the 8 cores (SPMD shape uniformity). Each
    128-token chunk is reduced to its 128 dst rows by a PE matmul with an
    on-device-built (one-hot x weight) matrix; chunks accumulate in PSUM.
  - Per layer, the full node table y = h @ W_rel (f32, node-major, HBM) is
    rebuilt with an AllGather; dma_gather pulls 256B rows from it through
    four 32768-row windows (int16 index space).
  - The layer-1 dense part (y1 = x@W_rel1, r1 = x@W_root1 + b1) runs on
    device from the quantized x shard.
"""

import os

import numpy as np

P = 128
D = 64
NCORES = 8
WIN = 32768


def _ru(x, m):
    return (x + m - 1) // m * m


# ---------------------------------------------------------------- host prep


def _preprocess(x, edge_index, edge_weight):
    import ml_dtypes

    N = x.shape[0]
    src = np.asarray(edge_index[0], dtype=np.int64)
    dst = np.asarray(edge_index[1], dtype=np.int64)
    w = np.asarray(edge_weight, dtype=np.float64)

    # dedup parallel edges (sum weights)
    key = dst * N + src
    ukey, inv = np.unique(key, return_inverse=True)
    uw = np.bincount(inv, weights=w).astype(np.float32)
    udst = (ukey // N).astype(np.int64)
    usrc = (ukey % N).astype(np.int64)

    deg = np.bincount(udst, minlength=N)

    B = -(-N // (NCORES * P))  # blocks per core
    Npad = NCORES * B * P
    NW = -(-Npad // WIN)

    # order nodes by degree desc; deal sorted 128-blocks round-robin to cores
    order = np.argsort(-deg, kind="stable")
    order_pad = np.concatenate([order, np.full(Npad - N, -1, dtype=np.int64)])
    t = np.arange(Npad)
    g = t // P
    p = t % P
    c = g % NCORES
    j = g // NCORES
    newpos_of_sorted = c * (B * P) + j * P + p
    perm = np.empty(Npad, dtype=np.int64)
    perm[newpos_of_sorted] = order_pad
    sorted_pos = np.empty(N, dtype=np.int64)
    sorted_pos[order] = np.arange(N)
    inv_new = newpos_of_sorted[sorted_pos]

    nd = inv_new[udst]
    ns = inv_new[usrc]
    wid = ns // WIN
    ec = nd // (B * P)
    rem = nd % (B * P)
    ej = rem // P
    ep = rem % P

    # token counts per (core, block, window) -> padded to 128 over core-max
    cnt = np.zeros((NCORES, B, NW), dtype=np.int64)
    np.add.at(cnt, (ec, ej, wid), 1)
    Kp = _ru(cnt.max(axis=0), 128)  # [B, NW]
    Kp[:, 0] = np.maximum(Kp[:, 0], 128)
    if Kp.sum() // 128 % 2 == 1:  # keep total chunk count even (alignment)
        Kp[B - 1, 0] += 128
    TK = Kp.sum(axis=1)  # [B] tokens per block
    g0 = np.concatenate([[0], np.cumsum(TK)])  # block token offsets
    off_jw = np.concatenate(
        [np.zeros((B, 1), np.int64), np.cumsum(Kp, axis=1)], axis=1
    ) + g0[:-1, None]
    T = int(g0[-1])
    Cj = TK // 128
    chunk0 = g0 // 128
    C_tot = T // 128

    # slot assignment: edges sorted by (core, block, window), dense packing
    ekey = (ec * B + ej) * NW + wid
    eorder = np.argsort(ekey, kind="stable")
    ek_s = ekey[eorder]
    first = np.concatenate([[True], ek_s[1:] != ek_s[:-1]])
    gid = np.cumsum(first) - 1
    gstart = np.nonzero(first)[0]
    k_within = np.arange(len(ek_s)) - gstart[gid]
    ec_s = ec[eorder]
    slot = off_jw[ej[eorder], wid[eorder]] + k_within

    tok_src = np.zeros((NCORES, T), dtype=np.int16)
    tok_dst = np.zeros((NCORES, T), dtype=np.int8)
    tok_w = np.zeros((NCORES, T), dtype=np.float32)
    tok_src[ec_s, slot] = (ns[eorder] % WIN).astype(np.int16)
    tok_dst[ec_s, slot] = ep[eorder].astype(np.int8)
    tok_w[ec_s, slot] = uw[eorder]

    # wrapped [16, T/16] dma_gather token table (per (j, w) range)
    idx16 = np.zeros((NCORES, 16, T // 16), dtype=np.int16)
    for jb in range(B):
        for wnd in range(NW):
            ntok = int(Kp[jb, wnd])
            if ntok == 0:
                continue
            a = int(off_jw[jb, wnd])
            lin = tok_src[:, a : a + ntok]
            idx16[:, :, a // 16 : (a + ntok) // 16] = lin.reshape(
                NCORES, ntok // 16, 16
            ).transpose(0, 2, 1)

    # chunk-major [128, C_tot] per-token dst / weight arrays
    dst_arr = np.ascontiguousarray(
        tok_dst.reshape(NCORES, C_tot, P).transpose(0, 2, 1)
    )
    w_arr = np.ascontiguousarray(
        tok_w.reshape(NCORES, C_tot, P).transpose(0, 2, 1)
    ).astype(ml_dtypes.bfloat16)

    # x shard: int8 row-quantized, node-major new layout
    xf = np.asarray(x, dtype=np.float32)
    x_new = np.zeros((Npad, D), dtype=np.float32)
    real = perm >= 0
    x_new[real] = xf[perm[real]]
    x_sh = x_new.reshape(NCORES, B * P, D)
    xs_row = np.maximum(np.abs(x_sh).max(axis=2), 1e-20) / 127.0
    x_q = np.rint(x_sh / xs_row[:, :, None]).astype(np.int8)
    xs = np.ascontiguousarray(xs_row.reshape(NCORES, B, P).transpose(0, 2, 1))

    return dict(
        N=N,
        B=B,
        Npad=Npad,
        NW=NW,
        perm=perm,
        Kp=Kp,
        off_jw=off_jw,
        g0=g0,
        Cj=Cj,
        chunk0=chunk0,
        C_tot=C_tot,
        T=T,
        idx16=idx16,
        dst_arr=dst_arr,
        w_arr=w_arr,
        x_q=x_q,
        xs=xs,
    )


def _pack_aux(prep, inputs):
    """Assemble the single [1, AUXH] int16 input per core.

    Byte layout (every region even-aligned, row sizes even):
      idx16   [16, T/16] i16      2T
      w_arr   [128, C_tot] bf16   2T
      xs      [128, B] f32        512B
      mats    6x[64,64] + 3x[1,64] + iota[1,128] f32
      x_q     [B*P, 64] i8        B*P*64
      dst_arr [128, C_tot] i8     T
    """
    B = prep["B"]
    T = prep["T"]

    mats = np.concatenate(
        [
            np.asarray(inputs[k], dtype=np.float32).ravel()
            for k in ("W_rel1", "W_root1", "W_rel2", "W_root2", "W_rel3", "W_root3",
                      "b_rel1", "b_rel2", "b_rel3")
        ]
        + [np.arange(P, dtype=np.float32)]
    )

    offs = {}
    pos = 0

    def reg(name, nbytes):
        nonlocal pos
        offs[name] = pos
        pos += nbytes

    reg("idx", 2 * T)
    reg("w", 2 * T)
    reg("xs", 4 * P * B)
    reg("mats", 4 * mats.size)
    reg("xq", B * P * D)
    reg("dst", T)
    auxh = _ru(pos, 2)

    aux = np.zeros((NCORES, auxh), dtype=np.int8)
    for c in range(NCORES):
        aux[c, offs["idx"] : offs["idx"] + 2 * T] = (
            prep["idx16"][c].ravel().view(np.int8)
        )
        aux[c, offs["w"] : offs["w"] + 2 * T] = prep["w_arr"][c].ravel().view(np.int8)
        aux[c, offs["xs"] : offs["xs"] + 4 * P * B] = (
            prep["xs"][c].ravel().view(np.int8)
        )
        aux[c, offs["mats"] : offs["mats"] + 4 * mats.size] = mats.view(np.int8)
        aux[c, offs["xq"] : offs["xq"] + B * P * D] = prep["x_q"][c].ravel().view(
            np.int8
        )
        aux[c, offs["dst"] : offs["dst"] + T] = prep["dst_arr"][c].ravel().view(
            np.int8
        )
    return aux.view(np.int16).reshape(NCORES, 1, auxh // 2), offs, mats.size


# ---------------------------------------------------------------- bass build


def _build(prep, offs, nmats):
    import concourse.bacc as bacc
    import concourse.mybir as mybir
    import concourse.tile as tile
    from concourse.masks import make_identity

    f32 = mybir.dt.float32
    i16 = mybir.dt.int16
    i8 = mybir.dt.int8
    bf16 = mybir.dt.bfloat16
    B = prep["B"]
    Npad = prep["Npad"]
    NW = prep["NW"]
    Kp = prep["Kp"]
    off_jw = prep["off_jw"]
    g0 = prep["g0"]
    Cj = prep["Cj"]
    chunk0 = prep["chunk0"]
    C_tot = prep["C_tot"]
    T = prep["T"]
    AUXH = None

    nc = bacc.Bacc(
        "TRN2",
        target_bir_lowering=False,
        debug=False,
        num_devices=NCORES,
    )

    auxh2 = (offs["dst"] + T + 1) // 2
    aux_in = nc.dram_tensor("aux", [1, auxh2], i16, kind="ExternalInput")
    OUTN = B * P * D + 4 * P * B
    out_t = nc.dram_tensor("out", [1, OUTN], i8, kind="ExternalOutput")

    def aux_ap(byte_off, n16, rows, cols16):
        assert rows * cols16 == n16
        return (
            aux_in.ap()[0:1, byte_off // 2 : byte_off // 2 + n16].rearrange(
                "o (p f) -> (o p) f", f=cols16
            )
        )

    with tile.TileContext(nc) as tc:
        with (
            tc.tile_pool(name="const", bufs=1) as cpool,
            tc.tile_pool(name="dram", bufs=1, space="DRAM") as dpool,
            tc.tile_pool(name="gather", bufs=4) as gpool,
            tc.tile_pool(name="oh", bufs=2) as opool,
            tc.tile_pool(name="work", bufs=4) as wpool,
            tc.tile_pool(name="psum", bufs=1, space="PSUM") as ppool,
        ):
            # ---- residents
            idx_res = cpool.tile([P, T // 16], i16, tag="idx")
            w_res = cpool.tile([P, C_tot], f32, tag="w")
            dst_res = cpool.tile([P, C_tot], f32, tag="dst")
            r_res = cpool.tile([P, B * D], f32, tag="r")
            xs_res = cpool.tile([P, B], f32, tag="xs")
            os_res = cpool.tile([P, B], f32, tag="os")
            iot = cpool.tile([P, P], f32, tag="iot")
            ident = cpool.tile([P, P], f32, tag="ident")

            # token index table: [16, T/16] replicated to 128 partitions
            for k8 in range(8):
                nc.sync.dma_start(
                    out=idx_res[16 * k8 : 16 * (k8 + 1), :],
                    in_=aux_ap(offs["idx"], T, 16, T // 16),
                )
            # weights bf16 -> f32
            w_st = cpool.tile([P, C_tot], i16, tag="wst")
            nc.sync.dma_start(out=w_st[:], in_=aux_ap(offs["w"], T, P, C_tot))
            nc.vector.tensor_copy(out=w_res[:], in_=w_st[:].bitcast(bf16))
            # dst ids i8 -> f32
            d_st = cpool.tile([P, C_tot // 2], i16, tag="dst_st")
            nc.sync.dma_start(out=d_st[:], in_=aux_ap(offs["dst"], T // 2, P, C_tot // 2))
            nc.vector.tensor_copy(out=dst_res[:], in_=d_st[:].bitcast(i8))
            # xs f32
            xs_st = cpool.tile([P, 2 * B], i16, tag="xs_st")
            nc.sync.dma_start(out=xs_st[:], in_=aux_ap(offs["xs"], 2 * P * B, P, 2 * B))
            nc.vector.tensor_copy(out=xs_res[:], in_=xs_st[:].bitcast(f32))
            # unpack mats into 2D tiles (i16 staging + SBUF-side bitcast)
            Wt = {}
            mp = 0
            for k in ("W_rel1", "W_root1", "W_rel2", "W_root2", "W_rel3", "W_root3"):
                w_stage = cpool.tile([D, 2 * D], i16, tag=k + "st", name=k + "st")
                nc.sync.dma_start(
                    out=w_stage[:],
                    in_=aux_ap(offs["mats"] + 4 * mp, 2 * D * D, D, 2 * D),
                )
                Wt[k] = cpool.tile([D, D], f32, tag=k, name=k)
                nc.vector.tensor_copy(out=Wt[k][:], in_=w_stage[:].bitcast(f32))
                mp += D * D
            b_bc = {}
            for k in ("b1", "b2", "b3"):
                b_stage = cpool.tile([1, 2 * D], i16, tag=k + "st", name=k + "st")
                nc.sync.dma_start(
                    out=b_stage[:],
                    in_=aux_ap(offs["mats"] + 4 * mp, 2 * D, 1, 2 * D),
                )
                b_row = cpool.tile([1, D], f32, tag=k, name=k)
                nc.vector.tensor_copy(out=b_row[:], in_=b_stage[:].bitcast(f32))
                b_bc[k] = cpool.tile([P, D], f32, tag=k + "bc", name=k + "bc")
                nc.gpsimd.partition_broadcast(b_bc[k][:], b_row[:])
                mp += D
            i_stage = cpool.tile([1, 2 * P], i16, tag="iotst")
            nc.sync.dma_start(
                out=i_stage[:],
                in_=aux_ap(offs["mats"] + 4 * mp, 2 * P, 1, 2 * P),
            )
            iot_row = cpool.tile([1, P], f32, tag="iotrow")
            nc.vector.tensor_copy(out=iot_row[:], in_=i_stage[:].bitcast(f32))
            nc.gpsimd.partition_broadcast(iot[:], iot_row[:])
            make_identity(nc, ident[:])

            # DRAM: ping-pong table + own-shard staging
            table2 = dpool.tile([Npad, D], f32, tag="table")
            y_own = dpool.tile([B * P, D], f32, tag="yown")

            # ---- layer-0 dense pass: y_own = x @ W_rel1, r = x @ W_root1
            for jb in range(B):
                xq_st = wpool.tile([P, D // 2], i16, tag="xq_st")
                nc.sync.dma_start(
                    out=xq_st[:],
                    in_=aux_ap(offs["xq"] + jb * P * D, P * D // 2, P, D // 2),
                )
                x_f = wpool.tile([P, D], f32, tag="xf")
                nc.vector.tensor_scalar(
                    out=x_f[:],
                    in0=xq_st[:].bitcast(i8),
                    scalar1=xs_res[:, jb : jb + 1],
                    scalar2=None,
                    op0=mybir.AluOpType.mult,
                )
                xTp = ppool.tile([D, P], f32, tag="hTp", bufs=2)
                nc.tensor.transpose(out=xTp[:], in_=x_f[:], identity=ident[:])
                xT = wpool.tile([D, P], f32, tag="hT")
                nc.scalar.activation(
                    out=xT[:], in_=xTp[:], func=mybir.ActivationFunctionType.Copy
                )
                yp = ppool.tile([P, D], f32, tag="yp", bufs=2)
                nc.tensor.matmul(
                    out=yp[:], lhsT=xT[:], rhs=Wt["W_rel1"][:], start=True, stop=True
                )
                rp = ppool.tile([P, D], f32, tag="rp", bufs=2)
                nc.tensor.matmul(
                    out=rp[:], lhsT=xT[:], rhs=Wt["W_root1"][:], start=True, stop=True
                )
                y_s = wpool.tile([P, D], f32, tag="y_s")
                nc.scalar.activation(
                    out=y_s[:], in_=yp[:], func=mybir.ActivationFunctionType.Copy
                )
                nc.vector.tensor_copy(out=r_res[:, jb * D : (jb + 1) * D], in_=rp[:])
                nc.sync.dma_start(out=y_own[jb * P : (jb + 1) * P, :], in_=y_s[:])
            nc.vector.tensor_tensor(
                out=r_res[:].rearrange("p (j f) -> p j f", f=D),
                in0=r_res[:].rearrange("p (j f) -> p j f", f=D),
                in1=b_bc["b1"][:].unsqueeze(1).to_broadcast([P, B, D]),
                op=mybir.AluOpType.add,
            )

            # ---- three aggregation layers
            for layer in (1, 2, 3):
                nc.gpsimd.collective_compute(
                    "AllGather",
                    mybir.AluOpType.bypass,
                    replica_groups=[list(range(NCORES))],
                    ins=[y_own[:].opt()],
                    outs=[table2[:].opt()],
                )
                W_rel_nxt = Wt[f"W_rel{layer + 1}"] if layer < 3 else None
                W_root_nxt = Wt[f"W_root{layer + 1}"] if layer < 3 else None

                for jb in range(B):
                    C = int(Cj[jb])
                    c0 = int(chunk0[jb])
                    g = gpool.tile([P, C * D], f32, tag="g")
                    for wnd in range(NW):
                        ntok = int(Kp[jb, wnd])
                        if ntok == 0:
                            continue
                        a = int(off_jw[jb, wnd])
                        cw = (a - int(g0[jb])) // P
                        r0 = wnd * WIN
                        r1 = min(Npad, (wnd + 1) * WIN)
                        nc.gpsimd.dma_gather(
                            out_ap=g[
                                :, cw * D : (cw + ntok // P) * D
                            ].rearrange("p (c e) -> p c e", e=D),
                            in_ap=table2[:][r0:r1, :],
                            idxs_ap=idx_res[:, a // 16 : (a + ntok) // 16],
                            num_idxs=ntok,
                            num_idxs_reg=ntok,
                            elem_size=D,
                            single_packet=False,
                        )
                    # one-hot x weight matrices for all chunks of this block
                    oh = opool.tile([P, C * P], f32, tag="oh")
                    oh3 = oh[:].rearrange("p (c d) -> p c d", d=P)
                    nc.vector.tensor_tensor(
                        out=oh3,
                        in0=dst_res[:, c0 : c0 + C].unsqueeze(-1).to_broadcast(
                            [P, C, P]
                        ),
                        in1=iot[:].unsqueeze(1).to_broadcast([P, C, P]),
                        op=mybir.AluOpType.is_equal,
                    )
                    nc.vector.tensor_tensor(
                        out=oh3,
                        in0=oh3,
                        in1=w_res[:, c0 : c0 + C].unsqueeze(-1).to_broadcast(
                            [P, C, P]
                        ),
                        op=mybir.AluOpType.mult,
                    )
                    agg_p = ppool.tile([P, D], f32, tag="aggp", bufs=2)
                    for ch in range(C):
                        nc.tensor.matmul(
                            out=agg_p[:],
                            lhsT=oh[:, ch * P : (ch + 1) * P],
                            rhs=g[:, ch * D : (ch + 1) * D],
                            start=(ch == 0),
                            stop=(ch == C - 1),
                        )
                    pre = wpool.tile([P, D], f32, tag="pre")
                    nc.vector.tensor_add(
                        out=pre[:],
                        in0=agg_p[:],
                        in1=r_res[:, jb * D : (jb + 1) * D],
                    )

                    if layer == 3:
                        am = wpool.tile([P, 1], f32, tag="am")
                        nc.vector.reduce_max(
                            out=am[:],
                            in_=pre[:],
                            axis=mybir.AxisListType.X,
                            apply_absolute_value=True,
                        )
                        nc.vector.tensor_scalar_max(am[:], am[:], 1e-20)
                        inv = wpool.tile([P, 1], f32, tag="inv")
                        nc.vector.reciprocal(out=inv[:], in_=am[:])
                        nc.vector.tensor_scalar_mul(inv[:], inv[:], 127.0)
                        o_q = wpool.tile([P, D], i8, tag="oq")
                        nc.vector.tensor_scalar(
                            out=o_q[:],
                            in0=pre[:],
                            scalar1=inv[:, 0:1],
                            scalar2=None,
                            op0=mybir.AluOpType.mult,
                        )
                        nc.vector.tensor_scalar_mul(
                            os_res[:, jb : jb + 1], am[:], 1.0 / 127.0
                        )
                        nc.sync.dma_start(
                            out=out_t.ap()[0:1, jb * P * D : (jb + 1) * P * D]
                            .rearrange("o (p f) -> (o p) f", f=D),
                            in_=o_q[:],
                        )
                        continue

                    h = wpool.tile([P, D], f32, tag="h")
                    nc.scalar.activation(
                        out=h[:], in_=pre[:], func=mybir.ActivationFunctionType.Relu
                    )
                    hTp = ppool.tile([D, P], f32, tag="hTp", bufs=2)
                    nc.tensor.transpose(out=hTp[:], in_=h[:], identity=ident[:])
                    hT = wpool.tile([D, P], f32, tag="hT")
                    nc.scalar.activation(
                        out=hT[:], in_=hTp[:], func=mybir.ActivationFunctionType.Copy
                    )
                    yp = ppool.tile([P, D], f32, tag="yp", bufs=2)
                    nc.tensor.matmul(
                        out=yp[:], lhsT=hT[:], rhs=W_rel_nxt[:], start=True, stop=True
                    )
                    rp = ppool.tile([P, D], f32, tag="rp", bufs=2)
                    nc.tensor.matmul(
                        out=rp[:], lhsT=hT[:], rhs=W_root_nxt[:], start=True, stop=True
                    )
                    y_s = wpool.tile([P, D], f32, tag="y_s")
                    nc.scalar.activation(
                        out=y_s[:], in_=yp[:], func=mybir.ActivationFunctionType.Copy
                    )
                    nc.vector.tensor_copy(
                        out=r_res[:, jb * D : (jb + 1) * D], in_=rp[:]
                    )
                    nc.sync.dma_start(
                        out=y_own[jb * P : (jb + 1) * P, :], in_=y_s[:]
                    )

                if layer < 3:
                    nc.vector.tensor_tensor(
                        out=r_res[:].rearrange("p (j f) -> p j f", f=D),
                        in0=r_res[:].rearrange("p (j f) -> p j f", f=D),
                        in1=b_bc[f"b{layer + 1}"][:]
                        .unsqueeze(1)
                        .to_broadcast([P, B, D]),
                        op=mybir.AluOpType.add,
                    )

            nc.sync.dma_start(
                out=out_t.ap()[0:1, B * P * D : B * P * D + 4 * P * B].rearrange(
                    "o (p f) -> (o p) f", f=4 * B
                ),
                in_=os_res[:].bitcast(i8),
            )

    nc.compile()
    return nc


# ---------------------------------------------------------------- entry


def _prep_and_build(inputs):
    prep = _preprocess(inputs["x"], inputs["edge_index"], inputs["edge_weight"])
    aux, offs, nmats = _pack_aux(prep, inputs)
    nc = _build(prep, offs, nmats)
    in_maps = [{"aux": aux[c]} for c in range(NCORES)]
    return prep, nc, in_maps


def _reassemble(prep, core_outs):
    N = prep["N"]
    B = prep["B"]
    perm = prep["perm"]
    out = np.zeros((N, D), dtype=np.float32)
    for c in range(NCORES):
        flat = core_outs[c].reshape(-1)  # [OUTN] int8
        q = flat[: B * P * D].reshape(B * P, D).astype(np.float32)
        s = flat[B * P * D :].view(np.float32).reshape(P, B)
        vals = q * s.T.reshape(B * P, 1)
        pr = perm[c * B * P : (c + 1) * B * P]
        real = pr >= 0
        out[pr[real]] = vals[real]
    return out


def _make_runner(nc):
    """callable(in_maps) -> per-core output arrays, via PJRT on the 8 cores.

    Mirrors concourse.bass2jax.run_bass_via_pjrt's multi-core branch, except
    donated ExternalOutput zero-buffers are created ON DEVICE and output
    shards are fetched in parallel.
    """
    from concurrent.futures import ThreadPoolExecutor

    import jax
    import jax.numpy as jnp
    from jax.experimental.shard_map import shard_map
    from jax.sharding import Mesh, NamedSharding, PartitionSpec

    import concourse.mybir as mybir
    from concourse.bass2jax import (
        _bass_exec_p,
        install_neuronx_cc_hook,
        partition_id_tensor,
    )

    install_neuronx_cc_hook()
    n_cores = NCORES
    partition_name = nc.partition_id_tensor.name if nc.partition_id_tensor else None

    in_names = []
    out_names = []
    out_avals = []
    for alloc in nc.m.functions[0].allocations:
        if not isinstance(alloc, mybir.MemoryLocationSet):
            continue
        if alloc.kind not in ("ExternalInput", "ExternalOutput"):
            continue
        name = alloc.memorylocations[0].name
        if alloc.kind == "ExternalInput":
            if name != partition_name:
                in_names.append(name)
        else:
            out_names.append(name)
            out_avals.append(
                jax.core.ShapedArray(
                    tuple(alloc.tensor_shape), mybir.dt.np(alloc.dtype)
                )
            )
    n_params = len(in_names)
    n_outs = len(out_names)
    all_names = list(in_names) + out_names
    if partition_name is not None:
        all_names.append(partition_name)

    def _body(*args):
        operands = list(args)
        if partition_name is not None:
            operands.append(partition_id_tensor())
        return tuple(
            _bass_exec_p.bind(
                *operands,
                out_avals=tuple(out_avals),
                in_names=tuple(all_names),
                out_names=tuple(out_names),
                lowering_input_output_aliases=(),
                sim_require_finite=True,
                sim_require_nnan=True,
                nc=nc,
            )
        )

    devices = jax.devices()[:n_cores]
    mesh = Mesh(np.asarray(devices), ("core",))
    in_specs = (PartitionSpec("core"),) * (n_params + n_outs)
    out_specs = (PartitionSpec("core"),) * n_outs
    donate = tuple(range(n_params, n_params + n_outs))
    sharded = jax.jit(
        shard_map(
            _body, mesh=mesh, in_specs=in_specs, out_specs=out_specs, check_rep=False
        ),
        donate_argnums=donate,
        keep_unused=True,
    )
    zshard = NamedSharding(mesh, PartitionSpec("core"))

    def _mkzeros():
        return tuple(
            jnp.zeros((n_cores * av.shape[0], *av.shape[1:]), av.dtype)
            for av in out_avals
        )

    mkzeros = jax.jit(_mkzeros, out_shardings=(zshard,) * n_outs)
    pool = ThreadPoolExecutor(n_cores)
    timeit = bool(int(os.environ.get("GCN_TIMEIT", "0")))

    def _fetch(arr, nrows):
        """Parallel per-shard device->host fetch of a sharded global array."""
        parts = [None] * n_cores

        def one(sh):
            i = sh.index[0].start or 0
            parts[i // nrows] = np.asarray(sh.data)

        list(pool.map(one, arr.addressable_shards))
        return parts

    def run(in_maps):
        import time as _time

        t0 = _time.time()
        concat_in = [
            np.concatenate(
                [np.asarray(in_maps[c][name]) for c in range(n_cores)], axis=0
            )
            for name in in_names
        ]
        t1 = _time.time()
        zs = mkzeros()
        out_arrs = sharded(*concat_in, *zs)
        for o in out_arrs:
            o.block_until_ready()
        t2 = _time.time()
        fetched = [_fetch(o, out_avals[i].shape[0]) for i, o in enumerate(out_arrs)]
        t3 = _time.time()
        if timeit:
            print(
                f"[runner] concat {t1 - t0:.3f}s  dispatch+exec {t2 - t1:.3f}s  "
                f"fetch {t3 - t2:.3f}s"
            )
        return [
            {name: fetched[i][c] for i, name in enumerate(out_names)}
            for c in range(n_cores)
        ]

    return run


def kernel(**inputs) -> np.ndarray:
    prep, nc, in_maps = _prep_and_build(inputs)
    run = _make_runner(nc)
    results = run(in_maps)
    kernel.last_run = run
    kernel.last_nc = nc
    kernel.last_in_maps = in_maps
    return _reassemble(prep, [results[c]["out"] for c in range(NCORES)])


if __name__ == "__main__":
    import reference

    inputs = {k: np.asarray(v) for k, v in reference.setup_inputs().items()}
    expected = np.asarray(reference.reference(**inputs))
    actual = kernel(**inputs)
    err = np.abs(actual - expected).max() / (np.abs(expected).max() + 1e-9)
    rel = np.linalg.norm(actual - expected) / (np.linalg.norm(expected) + 1e-30)
    print("max-abs-rel:", err, " fro-rel:", rel)
